# revision 6
# baseline (speedup 1.0000x reference)
"""Batched matrix-attention scores kernel for Trainium2 (8 NeuronCores).

Computes scores[b, i, j] = sum_d m1[b, i, d] * m2[b, j, d]
  (i.e. jnp.einsum('bid,bjd->bij', matrix_1, matrix_2))
with B=16, R1=R2=2048, D=256, fp32 in/out.

Sharding: data-parallel over batch - 2 batches per core on 8 cores.

Per-core HBM traffic is 8 MiB of loads + 32 MiB of stores; a single
HWDGE queue sustains ~420 GB/s, so the roofline is ~100 us. The
schedule is built to keep the DMA queues fed continuously:

  - b0 loads are split across both HWDGE rings (sync+scalar) in
    512 KiB quad-chunks; b1 loads follow on the scalar ring while
    stores own the sync ring.
  - Operands are PE-transposed (matmul transpose mode with identity)
    into D-on-partitions layout mT[d, dc, row]; transposes are packed
    4-to-a-PSUM-bank so one [128,512] copy evacuates a whole quad, and
    quads chase the load chunks so the first matmul block starts ~8us.
  - Each 128-row output tile takes 8 matmuls (2 d-chunks x 4 j-chunks
    of N=512, two 2-bank PSUM tiles); PSUM is evacuated split across
    DVE and ACT (one [128,1024] copy each) so store production stays
    above the DMA drain rate; output stored in 1 MiB blocks on the
    sync ring.
  - Remaining transpose quads (m1 of the current batch, both matrices
    of the next) are trickled between matmul blocks so the PE never
    lets the store queue drain.

Operands use dt.float32r (fp32 bits, full-rate single-pass PE matmul;
~2^-11 input mantissa truncation). Accumulation stays fp32 in PSUM.
"""

from contextlib import ExitStack

import numpy as np

import concourse.bass as bass
import concourse.mybir as mybir
import concourse.tile as tile
from concourse import bacc
from concourse.bass_utils import run_bass_kernel_spmd

F32 = mybir.dt.float32
F32R = mybir.dt.float32r

NCORES = 8
B, R1, R2, D = 16, 2048, 2048, 256
BPC = B // NCORES  # batches per core
P = 128
NJ_TILE = 512  # matmul free dim (one fp32 PSUM bank)
NJ = R2 // NJ_TILE  # j-chunks per row-block
NT = R1 // P  # 128-row tiles per batch
DC = D // P  # contraction chunks
NQ = NT // 4  # transpose quads (4 row-blocks) per (matrix, dc)
WARMUP_T = 12  # HAM warmup transposes


def _build_tile_kernel(ctx: ExitStack, tc: tile.TileContext, m1, m2, ident_in, out):
    nc = tc.nc

    const_pool = ctx.enter_context(tc.tile_pool(name="const", bufs=1))
    ident = const_pool.tile([P, P], F32R)
    nc.sync.dma_start(ident, ident_in)

    nat_pool = ctx.enter_context(tc.tile_pool(name="nat", bufs=1))
    mt_pool = ctx.enter_context(tc.tile_pool(name="mt", bufs=1))
    tpsum = ctx.enter_context(tc.tile_pool(name="tpsum", bufs=2, space="PSUM"))
    mpsum = ctx.enter_context(tc.tile_pool(name="mpsum", bufs=3, space="PSUM"))
    outp = ctx.enter_context(tc.tile_pool(name="outp", bufs=6))

    nat = {}
    mt = {}
    for b in range(BPC):
        for name in ("m2", "m1"):
            nat[(name, b)] = nat_pool.tile(
                [P, NT, D], F32R, tag=f"nat_{name}_{b}", name=f"nat_{name}_{b}"
            )
            mt[(name, b)] = mt_pool.tile(
                [P, DC, R1], F32R, tag=f"mt_{name}_{b}", name=f"mt_{name}_{b}"
            )

    def load_chunk(eng, name, b, q):
        """One 512 KiB quad-chunk (4 row-blocks) of a matrix into nat."""
        src = m2 if name == "m2" else m1
        eng.dma_start(
            nat[(name, b)][:, q * 4 : (q + 1) * 4, :],
            src[b].rearrange("(o p) d -> p o d", p=P)[:, q * 4 : (q + 1) * 4, :],
        )

    t_toggle = [0]

    def t_quad(name, b, q, dc):
        """Transpose 4 row-blocks (one d-chunk) into one PSUM bank, then
        evacuate with a single [128,512] copy on alternating engines."""
        tp = tpsum.tile([P, NJ_TILE], F32R, tag="tp", name=f"tp_{name}_{b}_{q}_{dc}")
        for k in range(4):
            o = q * 4 + k
            nc.tensor.transpose(
                tp[:, k * P : (k + 1) * P],
                nat[(name, b)][:, o, dc * P : (dc + 1) * P],
                ident,
            )
        dst = mt[(name, b)][:, dc, q * NJ_TILE : (q + 1) * NJ_TILE]
        if t_toggle[0] % 2 == 0:
            nc.vector.tensor_copy(dst, tp)
        else:
            nc.scalar.copy(dst, tp)
        t_toggle[0] += 1

    def t_pair(name, b, q):
        for dc in range(DC):
            t_quad(name, b, q, dc)

    stages = {}

    def mm_half(b, it, half, split_store=False):
        """Half of a 128-row output tile (j-halves of 1024): 4 matmuls into
        one 2-bank PSUM tile, evacuated on DVE (half 0) / ACT (half 1).
        split_store: store this half on its own (512 KiB); else the full
        1 MiB row-block is stored on sync once both halves are staged."""
        m2T, m1T = mt[("m2", b)], mt[("m1", b)]
        if (b, it) not in stages:
            stages[(b, it)] = outp.tile(
                [P, R2], F32, tag="stage", name=f"stage_{b}_{it}"
            )
        stage = stages[(b, it)]
        ps = mpsum.tile([P, 2 * NJ_TILE], F32, tag="mm", name=f"mps_{b}_{it}_{half}")
        for jl in range(2):
            jc = half * 2 + jl
            for dc in range(DC):
                nc.tensor.matmul(
                    ps[:, jl * NJ_TILE : (jl + 1) * NJ_TILE],
                    m1T[:, dc, it * P : (it + 1) * P],
                    m2T[:, dc, jc * NJ_TILE : (jc + 1) * NJ_TILE],
                    start=(dc == 0),
                    stop=(dc == DC - 1),
                )
        lo, hi = half * 2 * NJ_TILE, (half + 1) * 2 * NJ_TILE
        dst = stage[:, lo:hi]
        if half == 0:
            nc.vector.tensor_copy(dst, ps)
        else:
            nc.scalar.copy(dst, ps)
        if split_store:
            eng = nc.scalar if (half == 1 and it == NT - 1 and b == BPC - 1) else nc.sync
            eng.dma_start(
                out[b, it * P : (it + 1) * P, lo:hi], stage[:, lo:hi]
            )
        elif half == 1:
            nc.sync.dma_start(out[b, it * P : (it + 1) * P, :], stage)

    def mm_block(b, it, split_store=False):
        mm_half(b, it, 0, split_store)
        mm_half(b, it, 1, split_store)

    # ---- loads ----
    # b0 is ramp-critical: split across sync+scalar rings with the
    # first-store-critical chunks (m2 q0/q1, m1 q0) in front. All b1
    # loads ride the otherwise-idle gpsimd (SWDGE) queue, held back by
    # tiny copies that depend on the last b0 chunks of each ring, so b1
    # doesn't steal ramp bandwidth from b0.
    for name, b, q in (("m2", 0, 0), ("m2", 0, 2), ("m1", 0, 1), ("m1", 0, 3)):
        load_chunk(nc.sync, name, b, q)
    for name, b, q in (("m2", 0, 1), ("m1", 0, 0), ("m2", 0, 3), ("m1", 0, 2)):
        load_chunk(nc.scalar, name, b, q)
    gp_scratch = const_pool.tile([P, 4], F32R, tag="gps", name="gp_scratch")
    nc.gpsimd.tensor_copy(gp_scratch, nat[("m1", 0)][:, 11, 0:4])
    nc.gpsimd.tensor_copy(gp_scratch, nat[("m1", 0)][:, NT - 1, 0:4])
    for q in range(NQ):
        load_chunk(nc.gpsimd, "m2", 1, q)
    for q in range(NQ):
        load_chunk(nc.gpsimd, "m1", 1, q)

    # ---- HAM warmup: dummy transposes on the identity ----
    for w in range(WARMUP_T):
        wtp = tpsum.tile([P, NJ_TILE], F32R, tag="tp", name=f"warm_{w}")
        nc.tensor.transpose(wtp[:, 0:P], ident, ident)

    # ---- ramp: transposes chase the load chunks; blocks 0-1 run and
    # store in j-halves so the first store needs only half of m2T ----
    t_pair("m2", 0, 0)
    t_pair("m2", 0, 1)
    t_pair("m1", 0, 0)
    mm_half(0, 0, 0, split_store=True)
    mm_half(0, 1, 0, split_store=True)
    t_pair("m2", 0, 2)
    t_pair("m2", 0, 3)
    mm_half(0, 0, 1, split_store=True)
    mm_half(0, 1, 1, split_store=True)
    t_pair("m1", 0, 1)

    # schedule: after b0 block k, emit these transpose pairs/quads
    after_b0 = {
        3: [("m1", 0, 2)],
        5: [("m1", 0, 3)],
    }
    # b1 m2 quads (8 singles) after b0 blocks 6..13
    b1_m2 = [("m2", 1, q, dc) for q in range(NQ) for dc in range(DC)]
    after_b0_late = {
        14: [("m1", 1, 0)],
        15: [("m1", 1, 1)],
    }
    after_b1_blocks = {
        1: [("m1", 1, 2)],
        3: [("m1", 1, 3)],
    }

    for it in range(2, NT):
        mm_block(0, it)
        for name, b, q in after_b0.get(it, []):
            t_pair(name, b, q)
        if 6 <= it <= 13:
            name, b, q, dc = b1_m2[it - 6]
            t_quad(name, b, q, dc)
        for name, b, q in after_b0_late.get(it, []):
            t_pair(name, b, q)

    for it in range(NT):
        # last row-block stores in halves on both rings to shorten the
        # final drain
        mm_block(1, it, split_store=(it == NT - 1))
        for name, b, q in after_b1_blocks.get(it, []):
            t_pair(name, b, q)


_NC_CACHE = None


def _build():
    global _NC_CACHE
    if _NC_CACHE is not None:
        return _NC_CACHE
    nc = bacc.Bacc(
        "TRN2", target_bir_lowering=False, debug=False, num_devices=NCORES
    )
    m1 = nc.dram_tensor("m1", [BPC, R1, D], F32R, kind="ExternalInput").ap()
    m2 = nc.dram_tensor("m2", [BPC, R2, D], F32R, kind="ExternalInput").ap()
    ident_in = nc.dram_tensor("ident", [P, P], F32R, kind="ExternalInput").ap()
    out = nc.dram_tensor("out", [BPC, R1, R2], F32, kind="ExternalOutput").ap()
    with tile.TileContext(nc) as tc:
        with ExitStack() as ctx:
            _build_tile_kernel(ctx, tc, m1, m2, ident_in, out)
    nc.compile()
    _NC_CACHE = nc
    return nc


def kernel(matrix_1: np.ndarray, matrix_2: np.ndarray, **run_kwargs) -> np.ndarray:
    m1 = np.ascontiguousarray(np.asarray(matrix_1, dtype=np.float32))
    m2 = np.ascontiguousarray(np.asarray(matrix_2, dtype=np.float32))
    assert m1.shape == (B, R1, D) and m2.shape == (B, R2, D)

    nc = _build()
    eye = np.eye(P, dtype=np.float32)
    in_maps = [
        {
            "m1": m1[i * BPC : (i + 1) * BPC],
            "m2": m2[i * BPC : (i + 1) * BPC],
            "ident": eye,
        }
        for i in range(NCORES)
    ]
    res = run_bass_kernel_spmd(
        nc, in_maps, core_ids=list(range(NCORES)), **run_kwargs
    )
    out = np.empty((B, R1, R2), dtype=np.float32)
    for i in range(NCORES):
        out[i * BPC : (i + 1) * BPC] = res.results[i]["out"]
    if run_kwargs:
        kernel.last_result = res
    return out
